# revision 4
# baseline (speedup 1.0000x reference)
"""Trainium2 Bass kernel for BQuantConv1d (binary-quantized linear layer).

Computation: out[t, f] = sum_x x[t, x] * W[f, x] + bias[f]
  where W[f, x] = sum_b scale[f, b] * (2*bit(binary[f, b, x//8], x%8) - 1)

Sharding across 8 NeuronCores: hybrid 2-way data-parallel over tokens
(8192 -> 2 x 4096) x 4-way tensor-parallel over output features
(4096 -> 4 x 1024). Each core:
  - unpacks its binary shard to {0,1} bits: per bit position a fused
    VectorE tensor_scalar (shift-right + and-1) with strided output, then a
    ScalarE copy casts the plane to bf16,
  - contracts the 8 planes with per-(f,b) scales on the TensorEngine via
    diagonal matmuls accumulating in PSUM (diag(2*scale[:,b])), plus a 9th
    all-ones plane scaled by diag(-sum_b scale) for the -1 offset,
  - PE-transposes the resulting W[f, x] to WT[x, f],
  - streams x tiles with a casting DMA (f32 -> bf16), DMA-transposes them,
    and runs the main matmul out = xT.T @ WT accumulating over the 4096-dim
    contraction in PSUM,
  - adds broadcast bias on PSUM evacuation and DMAs the result out.
"""

from contextlib import ExitStack

import numpy as np

P = 128
BITS = 8
NX = 4096
NB = NX // 8           # packed bytes per feature
NF = 4096
NTOK = 8192            # 4 * 2048
TSHARD = 2             # data-parallel ways (tokens)
FSHARD = 4             # tensor-parallel ways (features)
T_LOC = NTOK // TSHARD     # 4096
NF_LOC = NF // FSHARD      # 1024

MASKV = np.array([128, 64, 32, 16, 8, 4, 2, 1], dtype=np.uint8)

# halves of the W build (PSUM footprint = NX/WHALVES * 4B per partition)
WHALVES = 2


def bass_body(ctx: ExitStack, tc, outs, ins, t_loc=T_LOC, nf_loc=NF_LOC):
    import concourse.bass as bass  # noqa: F401
    from concourse import mybir
    from concourse.masks import make_identity

    nc = tc.nc
    dt = mybir.dt
    NFT = nf_loc // P          # f-tiles
    NTT = t_loc // P           # t-tiles
    NKC = NX // P              # 32 contraction chunks
    XH = NX // WHALVES         # x columns per W-build half
    BH = NB // WHALVES         # bytes per W-build half
    NWC = XH // 512            # psum-bank chunks per half

    const = ctx.enter_context(tc.tile_pool(name="const", bufs=1))
    wpool = ctx.enter_context(tc.tile_pool(name="wpool", bufs=1))

    # ---- constants
    ident = const.tile([P, P], dt.bfloat16)
    make_identity(nc, ident[:])
    ones = const.tile([P, XH], dt.bfloat16)
    nc.gpsimd.memset(ones[:], 1.0)

    # bias -> [128, nf_loc] broadcast
    bias_row = const.tile([1, nf_loc], dt.float32)
    nc.sync.dma_start(bias_row[:], ins["bias_loc"][:].rearrange("(o f) -> o f", o=1))
    bias_bc = const.tile([P, nf_loc], dt.float32)
    nc.gpsimd.partition_broadcast(bias_bc[:], bias_row[:])

    # ---- phase A: build WT [x-part, kchunk, f] bf16
    wT = wpool.tile([P, NKC, nf_loc], dt.bfloat16)
    with (
        tc.tile_pool(name="wtmp", bufs=2) as wtmp,
        tc.tile_pool(name="wsc", bufs=2) as wsc,
        tc.tile_pool(name="bitp", bufs=3) as bitp,
        tc.tile_pool(name="psw", bufs=1, space="PSUM") as psw,
        tc.tile_pool(name="pst", bufs=2, space="PSUM") as pst,
    ):
        for ft in range(NFT):
            vt = wtmp.tile([P, BITS, NB], dt.uint8, tag="vt")
            nc.sync.dma_start(
                vt[:], ins["binary_loc"][:].rearrange("(a p) b j -> a p b j", p=P)[ft]
            )
            st = wsc.tile([P, BITS], dt.float32, tag="st")
            nc.sync.dma_start(
                st[:], ins["scale_loc"][:].rearrange("(a p) b -> a p b", p=P)[ft]
            )
            nssum = wsc.tile([P, 1], dt.float32, tag="nssum")
            nc.vector.tensor_reduce(
                out=nssum[:], in_=st[:], axis=mybir.AxisListType.X,
                op=mybir.AluOpType.add, negate=True,
            )
            diags = wsc.tile([P, BITS + 1, P], dt.bfloat16, tag="diags")
            for b in range(BITS):
                nc.vector.tensor_scalar(
                    out=diags[:, b, :], in0=ident[:], scalar1=st[:, b : b + 1],
                    scalar2=2.0, op0=mybir.AluOpType.mult, op1=mybir.AluOpType.mult,
                )
            nc.vector.tensor_scalar(
                out=diags[:, BITS, :], in0=ident[:], scalar1=nssum[:, 0:1],
                scalar2=None, op0=mybir.AluOpType.mult,
            )

            wm = wtmp.tile([P, NX], dt.bfloat16, tag="wm")
            for h in range(WHALVES):
                # unpack planes for this x-half, accumulate in PSUM
                wm_ps = psw.tile([P, XH], dt.float32)
                for b in range(BITS):
                    b8 = bitp.tile([P, XH], dt.uint8, tag="bits8")
                    for p in range(8):
                        # bit_p = (v >> (7-p)) & 1, strided out at [:, p::8]
                        nc.vector.tensor_scalar(
                            out=b8[:].rearrange("q (j i) -> q j i", i=8)[:, :, p],
                            in0=vt[:, b, h * BH : (h + 1) * BH],
                            scalar1=int(7 - p),
                            scalar2=int(1),
                            op0=mybir.AluOpType.logical_shift_right,
                            op1=mybir.AluOpType.bitwise_and,
                        )
                    bt = bitp.tile([P, XH], dt.bfloat16, tag="bits")
                    nc.scalar.copy(bt[:], b8[:])
                    for c in range(NWC):
                        nc.tensor.matmul(
                            wm_ps[:, c * 512 : (c + 1) * 512],
                            lhsT=diags[:, b, :],
                            rhs=bt[:, c * 512 : (c + 1) * 512],
                            start=(b == 0),
                            stop=False,
                        )
                for c in range(NWC):
                    nc.tensor.matmul(
                        wm_ps[:, c * 512 : (c + 1) * 512],
                        lhsT=diags[:, BITS, :],
                        rhs=ones[:, c * 512 : (c + 1) * 512],
                        start=False,
                        stop=True,
                    )
                nc.scalar.copy(wm[:, h * XH : (h + 1) * XH], wm_ps[:])

            # PE-transpose Wm -> wT[:, c, ft*128:(ft+1)*128]
            for c in range(NKC):
                tp = pst.tile([P, P], dt.bfloat16, tag="tp")
                nc.tensor.transpose(tp[:], wm[:, c * P : (c + 1) * P], ident[:])
                nc.scalar.copy(wT[:, c, ft * P : (ft + 1) * P], tp[:])

    # ---- phase B: stream x tiles, transpose, matmul
    NFH = nf_loc // 512        # psum-bank halves of the output tile
    with (
        tc.tile_pool(name="xpool", bufs=3) as xpool,
        tc.tile_pool(name="opool", bufs=2) as opool,
        tc.tile_pool(name="pso", bufs=2, space="PSUM") as pso,
    ):
        for ti in range(NTT):
            xr = xpool.tile([P, NX], dt.bfloat16, tag="xr")
            nc.gpsimd.dma_start(
                xr[:], ins["x_loc"][:].rearrange("(a p) x -> a p x", p=P)[ti]
            )
            xT = xpool.tile([P, NKC, P], dt.bfloat16, tag="xT")
            nc.sync.dma_start(xT[:], xr[:], transpose=True)

            ops = pso.tile([P, nf_loc], dt.float32, tag="ops")
            for k in range(NKC):
                for fh in range(NFH):
                    nc.tensor.matmul(
                        ops[:, fh * 512 : (fh + 1) * 512],
                        lhsT=xT[:, k, :],
                        rhs=wT[:, k, fh * 512 : (fh + 1) * 512],
                        start=(k == 0),
                        stop=(k == NKC - 1),
                    )
            out_sb = opool.tile([P, nf_loc], dt.float32, tag="out")
            nc.vector.tensor_tensor(
                out=out_sb[:], in0=ops[:], in1=bias_bc[:], op=mybir.AluOpType.add,
            )
            nc.sync.dma_start(
                outs["out_loc"][:].rearrange("(a p) f -> a p f", p=P)[ti], out_sb[:]
            )


def build_nc(t_loc=T_LOC, nf_loc=NF_LOC):
    from concourse import bacc, mybir
    import concourse.tile as tile

    dt = mybir.dt
    nc = bacc.Bacc("TRN2", target_bir_lowering=False, debug=False)
    ins = {
        "x_loc": nc.dram_tensor("x_loc", [t_loc, NX], dt.float32, kind="ExternalInput").ap(),
        "binary_loc": nc.dram_tensor("binary_loc", [nf_loc, BITS, NB], dt.uint8, kind="ExternalInput").ap(),
        "scale_loc": nc.dram_tensor("scale_loc", [nf_loc, BITS], dt.float32, kind="ExternalInput").ap(),
        "bias_loc": nc.dram_tensor("bias_loc", [nf_loc], dt.float32, kind="ExternalInput").ap(),
    }
    outs = {
        "out_loc": nc.dram_tensor("out_loc", [t_loc, nf_loc], dt.float32, kind="ExternalOutput").ap(),
    }
    with tile.TileContext(nc) as tc:
        with ExitStack() as ctx:
            bass_body(ctx, tc, outs, ins, t_loc=t_loc, nf_loc=nf_loc)
    nc.compile()
    return nc


def make_in_maps(x, scale, bias, binary):
    """Shard full inputs into the 8 per-core input maps."""
    xf = np.ascontiguousarray(np.asarray(x, dtype=np.float32).reshape(NTOK, NX))
    b8 = np.ascontiguousarray(
        np.asarray(binary).reshape(NF, BITS, NB).astype(np.uint8)
    )
    s2 = np.ascontiguousarray(np.asarray(scale, dtype=np.float32).reshape(NF, BITS))
    bb = np.ascontiguousarray(np.asarray(bias, dtype=np.float32))
    in_maps = []
    for core in range(TSHARD * FSHARD):
        g, c = divmod(core, FSHARD)
        in_maps.append(
            {
                "x_loc": xf[g * T_LOC : (g + 1) * T_LOC],
                "binary_loc": b8[c * NF_LOC : (c + 1) * NF_LOC],
                "scale_loc": s2[c * NF_LOC : (c + 1) * NF_LOC],
                "bias_loc": bb[c * NF_LOC : (c + 1) * NF_LOC],
            }
        )
    return in_maps


def assemble_output(results, out_shape=(4, 2048, NF)):
    out = np.empty((NTOK, NF), dtype=np.float32)
    for core in range(TSHARD * FSHARD):
        g, c = divmod(core, FSHARD)
        out[g * T_LOC : (g + 1) * T_LOC, c * NF_LOC : (c + 1) * NF_LOC] = results[
            core
        ]["out_loc"]
    return out.reshape(out_shape)


_NC_CACHE = {}


def _get_nc():
    if "nc" not in _NC_CACHE:
        _NC_CACHE["nc"] = build_nc()
    return _NC_CACHE["nc"]


def run_on_hw(x, scale, bias, binary, trace=False, **kwargs):
    from concourse.bass_utils import run_bass_kernel_spmd

    nc = _get_nc()
    in_maps = make_in_maps(x, scale, bias, binary)
    res = run_bass_kernel_spmd(
        nc, in_maps, core_ids=list(range(TSHARD * FSHARD)), trace=trace, **kwargs
    )
    return res


def kernel(x, scale, bias, binary):
    res = run_on_hw(x, scale, bias, binary, trace=False)
    return assemble_output(res.results, out_shape=np.asarray(x).shape[:-1] + (NF,))


if __name__ == "__main__":
    rng = np.random.default_rng(0)
    x = rng.standard_normal((4, 2048, NX), dtype=np.float32)
    scale = rng.random((NF, 1, BITS), dtype=np.float32)
    bias = rng.standard_normal(NF).astype(np.float32)
    binary = rng.integers(0, 256, size=(NF, BITS, NB, 1), dtype=np.int32).astype(np.int8)
    out = kernel(x, scale, bias, binary)
    print(out.shape, out.dtype)
